# revision 66
# baseline (speedup 1.0000x reference)
"""Bass/Trainium2 kernel for nn_ExaoneMoEAttention (sliding-window GQA attention).

Strategy (8 NeuronCores, tensor-parallel over heads for QKV+attention,
token-sharded o_proj via AllToAll).  All GEMMs bf16: fp8/DoubleRow fails the
2e-2 max-rel gate because quantization noise concentrates in the first ~64
token rows (attention there averages few keys, so |attn| ~ |v| and the 3%
fp8 mantissa error lands directly on the output rows that set max|out|).
The per-matmul issue floor is ~260ns regardless of dtype or size, so the
design minimizes matmul instruction COUNT and keeps every engine's queue
free of cross-engine blockers:
  - core c owns q heads 4c..4c+3 and kv head c (w_qkv column shard [4096, 768]).
  - Phase A (QKV proj, transposed): chunk-major over four 512-column hid
    chunks (double-buffered 4MB DMAs); per chunk, ci PAIRS run as two
    interleaved psum chains so matmul durations overlap.  Startup DMAs are
    split fine (first pair gates on ~2MB).  RMSNorm/RoPE post-ops trail in
    3 lag stages: ones-matmul partition-reduce of sq=ps^2/D (scale folded
    into the Square activation, eps dropped as negligible), sqrt on ACT,
    reciprocal_approx_fast on DVE, Copy-downcast on ACT, ones-matmul
    broadcast, rope as two half-tile muls with norm weight and softmax
    scale folded into host cos/sin tables.  v is PE-transposed.
  - Phase B attention: scores = kT-block.T @ qT (N=512) with the additive
    -30000 mask folded into the score psum group (a helper-engine mask
    serializes behind the blocking collectives); exp on ACT (no max
    subtraction, |score| <= sqrt(D)); softmax denominator accumulated as
    bf16 tensor_adds on the DVE and partition-reduced by a single
    ones-matmul per head-chunk in the (deferred) normalization; two GQA
    heads pipelined per group, 4-deep score lookahead.
  - A2A schedule: hf=0 exchanged as ONE merged 1MB AllToAll (hides under
    qc2/3); hf=1 split per head-pair, the first fired under qc3-hp2 so only
    a 512KB exchange remains after attention ends.  The o_proj tail opens
    with three hf=0 blocks to cover that drain.
  - o_proj: each core owns 256 token rows with all 32 heads; out =
    attnT_full.T @ w_o against streamed bf16 w_o (4MB chunks, bufs=3,
    alternating DMA queues, chunks 0..2 resident from attention so nothing
    reloads).  Tail blocks are emitted in PAIRS/TRIPLES sharing each
    atf-head stationary with interleaved psum chains.  Output is directly
    row-sharded - host assembly is pure concatenation.
  - a tiny dummy AllToAll at kernel start absorbs the ~50us core-start skew.

Measured on 8 axon-tunneled trn2 cores: 577us (baseline 717us), max rel err
0.0043 vs the fp32 reference (gate 2e-2).
"""

import ml_dtypes
import numpy as np

import concourse.bass as bass
import concourse.mybir as mybir
import concourse.tile as tile
from concourse import bacc
from concourse.bass_utils import run_bass_kernel_spmd

F32 = mybir.dt.float32
BF16 = mybir.dt.bfloat16
AF = mybir.ActivationFunctionType

N_CORES = 8
T = 2048
HID = 4096
H = 32
HKV = 8
D = 128
EPS = 1e-5
THETA = 1e6
WINDOW = 1024

HL = H // N_CORES          # 4 local q heads
NT = T // 128              # 16 s tiles
KO = HID // 128            # 32 k-subtiles in projection
QC = 512                   # q chunk in attention phase
N_QC = T // QC             # 4
THALF = T // 2
NECH = HID // QC           # 8 o_proj e-chunks

MASK_DELTAS = [0, -128, -256, -384, 640, 768, 896, 1024]
MASK_IDX = {d: i for i, d in enumerate(MASK_DELTAS)}

# c-tile order within the per-core w_qkv shard: q0..q3, k, v
N_CT = HL + 2


def _build():
    nc = bacc.Bacc(num_devices=N_CORES)

    hidT = nc.declare_dram_parameter("hidT", [128, N_QC, KO, QC], BF16, isOutput=False)
    wq = nc.declare_dram_parameter("wq", [128, N_CT, KO, 128], BF16, isOutput=False)
    csq = nc.declare_dram_parameter("csq", [128, T], BF16, isOutput=False)
    snq = nc.declare_dram_parameter("snq", [128, T], BF16, isOutput=False)
    csk = nc.declare_dram_parameter("csk", [128, T], BF16, isOutput=False)
    snk = nc.declare_dram_parameter("snk", [128, T], BF16, isOutput=False)
    maskm = nc.declare_dram_parameter("maskm", [128, len(MASK_DELTAS), QC], BF16, isOutput=False)
    wo = nc.declare_dram_parameter("wo", [128, H, HID], BF16, isOutput=False)
    onesd = nc.declare_dram_parameter("onesd", [128, 128], BF16, isOutput=False)
    oncd = nc.declare_dram_parameter("oncd", [128, 1], BF16, isOutput=False)
    identd = nc.declare_dram_parameter("identd", [128, 128], BF16, isOutput=False)
    out_p = nc.declare_dram_parameter("out", [2, 128, HID], BF16, isOutput=True)

    with tile.TileContext(nc) as tc:
        with (
            tc.tile_pool(name="pers", bufs=1) as pers,
            tc.tile_pool(name="dram0", bufs=1, space="DRAM") as dram0,
        ):
            qT = [pers.tile([128, T], BF16, name=f"qT{h}") for h in range(HL)]
            kT = pers.tile([128, T], BF16)
            vnat = pers.tile([128, NT, 128], BF16)
            attnT = pers.tile([128, HL, T], BF16)
            ones_sb = pers.tile([128, 128], BF16)
            nc.sync.dma_start(out=ones_sb[:], in_=onesd[:])
            onc_sb = pers.tile([128, 1], BF16)
            nc.sync.dma_start(out=onc_sb[:], in_=oncd[:])
            ident_sb = pers.tile([128, 128], BF16)
            nc.sync.dma_start(out=ident_sb[:], in_=identd[:])
            # attention masks preloaded on the idle gpsimd queue so the first
            # masked score matmul never waits on this 0.5MB
            mask_sb = pers.tile([128, len(MASK_DELTAS), QC], BF16)
            nc.gpsimd.dma_start(out=mask_sb[:], in_=maskm[:])

            # tiny dummy collective: absorbs multi-core start skew so the
            # real AllToAlls later don't pay it (first collective waits for
            # the slowest core).
            dumm_i = dram0.tile([N_CORES, 16], BF16, name="dummi")
            dumm_o = dram0.tile([N_CORES, 16], BF16, name="dummo")
            nc.sync.dma_start(out=dumm_i[0:1, :], in_=onesd[0:1, 0:16])
            nc.gpsimd.collective_compute(
                "AllToAll",
                mybir.AluOpType.bypass,
                replica_groups=[list(range(N_CORES))],
                ins=[dumm_i[:]],
                outs=[dumm_o[:]],
            )

            # ---------------- Phase A: transposed QKV projection --------------
            with (
                tc.tile_pool(name="tabp", bufs=1) as tabp,
                tc.tile_pool(name="hidp", bufs=1) as hidp,
                tc.tile_pool(name="wqp", bufs=1) as wqp,
                tc.tile_pool(name="stg", bufs=2) as stg,
                tc.tile_pool(name="projp", bufs=2, space="PSUM") as projp,
                tc.tile_pool(name="varp", bufs=1, space="PSUM") as varp,
                tc.tile_pool(name="bcp", bufs=1, space="PSUM") as bcp,
            ):
                # wq resident; hid double-buffered by 4MB column chunk so the
                # first matmul pair gates on wq[ci 0,1] + chunk 0 (~6MB).
                # ci PAIRS run as two interleaved psum chains per chunk so
                # consecutive matmul durations overlap.
                hid_sb = hidp.tile([128, 2, KO, QC], BF16, name="hid_sb")
                wq_sb = wqp.tile([128, N_CT, KO, 128], BF16, name="wq_sb")
                ci_pairs = ((0, 1), (HL, HL + 1), (2, 3))
                # priority order, split fine so the first matmul pair gates
                # on ~2MB (wq[0,1] ko 0..15 + hid chunk 0 ko 0..7) and the
                # ko loop chases the in-flight rest
                nc.scalar.dma_start(out=wq_sb[:, 0, 0:16], in_=wq[:, 0, 0:16])
                nc.scalar.dma_start(out=wq_sb[:, 1, 0:16], in_=wq[:, 1, 0:16])
                nc.sync.dma_start(out=hid_sb[:, 0, 0:8], in_=hidT[:, 0, 0:8])
                nc.scalar.dma_start(out=hid_sb[:, 0, 8:16], in_=hidT[:, 0, 8:16])
                nc.sync.dma_start(out=hid_sb[:, 0, 16:24], in_=hidT[:, 0, 16:24])
                nc.scalar.dma_start(out=wq_sb[:, 0, 16:32], in_=wq[:, 0, 16:32])
                nc.scalar.dma_start(out=wq_sb[:, 1, 16:32], in_=wq[:, 1, 16:32])
                nc.sync.dma_start(out=hid_sb[:, 0, 24:32], in_=hidT[:, 0, 24:32])
                for ci in (HL, HL + 1, 2, 3):
                    nc.scalar.dma_start(out=wq_sb[:, ci], in_=wq[:, ci])
                nc.sync.dma_start(out=hid_sb[:, 1], in_=hidT[:, 1])

                cs_sb = {}
                for nm, src in (("csq", csq), ("snq", snq), ("csk", csk), ("snk", snk)):
                    t_ = tabp.tile([128, T], BF16, name=nm)
                    nc.scalar.dma_start(out=t_[:], in_=src[:])
                    cs_sb[nm] = t_

                # posts run in 3 lag stages so no PE instruction ever waits
                # on ACT/DVE work emitted in the same flush (which stalls the
                # in-order PE queue): stage1 evacuates psum (ACT/DVE only),
                # stage2 does the var matmul + sqrt + recip, stage3 the
                # broadcast matmul + rope muls (all inputs one flush old).
                pend = {1: [], 2: [], 3: []}

                def flush_stages(keep):
                    for lag in (1, 2, 3):
                        while len(pend[lag]) > keep.get(lag, 0):
                            pend[lag].pop(0)()

                for cc in range(N_QC):
                    buf = cc % 2
                    cols = slice(cc * QC, (cc + 1) * QC)
                    for cia, cib in ci_pairs:
                        ps = {}
                        for i, ci in enumerate((cia, cib)):
                            ps[ci] = projp.tile([128, QC], F32, tag=f"pj{i}",
                                                name=f"ps{i}")
                        for ko in range(KO):
                            for ci in (cia, cib):
                                nc.tensor.matmul(
                                    ps[ci][:], wq_sb[:, ci, ko],
                                    hid_sb[:, buf, ko, :],
                                    start=(ko == 0), stop=(ko == KO - 1),
                                )

                        evs, sqs, rrs = {}, {}, {}
                        for ci in (cia, cib):
                            evs[ci] = stg.tile([128, QC], BF16, tag="ev",
                                               name="ev", bufs=6)
                            if ci != HL + 1:
                                sqs[ci] = stg.tile([128, QC], BF16, tag="sq",
                                                   name="sq", bufs=4)
                                rrs[ci] = stg.tile([1, QC], BF16, tag="rr",
                                                   name="rr", bufs=4)

                        def st1(ps=ps, evs=evs, sqs=sqs):
                            for ci, p in ps.items():
                                if ci in sqs:
                                    # scale folded in: sq = ps^2/D, so the
                                    # ones-matmul reduce gives mean(x^2)
                                    # directly (eps=1e-5 is negligible vs
                                    # var~1.6 and is dropped)
                                    nc.scalar.activation(
                                        sqs[ci][:], p[:], AF.Square,
                                        scale=D ** -0.5,
                                    )
                                nc.vector.tensor_copy(evs[ci][:], p[:])
                        pend[1].append(st1)

                        def st2(cc=cc, evs=evs, sqs=sqs, rrs=rrs):
                            for ci in evs:
                                if ci == HL + 1:
                                    for b in range(4):
                                        si = cc * 4 + b
                                        tp = varp.tile([128, 128], BF16, tag="vt",
                                                       name="tp", bufs=2)
                                        nc.tensor.transpose(
                                            tp[:], evs[ci][:, b * 128:(b + 1) * 128],
                                            ident_sb[:],
                                        )
                                        nc.vector.tensor_copy(vnat[:, si, :], tp[:])
                                    continue
                                vr = varp.tile([1, QC], F32, tag="var", name="vr", bufs=1)
                                nc.tensor.matmul(
                                    vr[:], onc_sb[:], sqs[ci][:],
                                    start=True, stop=True,
                                )
                                sdr = stg.tile([1, QC], F32, tag="sdr", name="sdr")
                                nc.scalar.activation(sdr[:], vr[:], AF.Sqrt)
                                rrf = stg.tile([1, QC], F32, tag="rrf", name="rrf")
                                nc.vector.reciprocal_approx_fast(rrf[:], sdr[:])
                                with nc.allow_low_precision(reason="bf16 rstd ok at 2e-2 gate"):
                                    nc.scalar.activation(rrs[ci][:], rrf[:], AF.Copy)
                        pend[2].append(st2)

                        def st3(cols=cols, evs=evs, rrs=rrs):
                            for ci in evs:
                                if ci == HL + 1:
                                    continue
                                if ci == HL:
                                    cs_t, sn_t, dst = cs_sb["csk"], cs_sb["snk"], kT
                                else:
                                    cs_t, sn_t, dst = cs_sb["csq"], cs_sb["snq"], qT[ci]
                                rbc = bcp.tile([128, QC], F32, tag="bc", name="rbc")
                                nc.tensor.matmul(
                                    rbc[:], ones_sb[0:1, :], rrs[ci][:],
                                    start=True, stop=True,
                                )
                                t1 = stg.tile([128, QC], BF16, tag="t1", name="t1")
                                nc.vector.tensor_mul(t1[:], evs[ci][:], cs_t[:, cols])
                                sw = stg.tile([128, QC], BF16, tag="sw", name="sw")
                                nc.vector.tensor_copy(sw[0:64, :], evs[ci][64:128, :])
                                nc.vector.tensor_copy(sw[64:128, :], evs[ci][0:64, :])
                                t2 = stg.tile([128, QC], BF16, tag="t2", name="t2")
                                nc.vector.tensor_mul(t2[:], sw[:], sn_t[:, cols])
                                nc.vector.tensor_add(t1[:], t1[:], t2[:])
                                nc.vector.tensor_mul(dst[:, cols], t1[:], rbc[:])
                        pend[3].append(st3)

                        flush_stages({1: 1, 2: 2, 3: 3})
                    if cc + 2 < N_QC:
                        # prefetch chunk cc+2 into the buffer chunk cc just
                        # finished reading (WAR on the emitted matmuls)
                        nc.sync.dma_start(out=hid_sb[:, buf], in_=hidT[:, cc + 2])
                flush_stages({})

            # ---------------- Phase B: attention + A2A + o_proj ---------------
            with (
                tc.tile_pool(name="pB", bufs=1) as pB,
                tc.tile_pool(name="wop", bufs=3) as wop,
                tc.tile_pool(name="exp", bufs=12) as exp_p,
                tc.tile_pool(name="accp", bufs=4) as accp,
                tc.tile_pool(name="stB", bufs=3) as stB,
                tc.tile_pool(name="ostg", bufs=2) as ostg_p,
                tc.tile_pool(name="dramB", bufs=1, space="DRAM") as dramB,
            ):
                atf = [pB.tile([128, H, 128], BF16, name=f"atf{h}") for h in range(2)]

                # hf=0: ONE merged A2A (all 4 local heads, 1MB) - it hides
                # fully under qc2/qc3 attention.  hf=1: split per head-pair so
                # the first half overlaps qc3-hp2 attention and only a small
                # 512KB exchange remains exposed after attention ends.
                a2a_in = {}
                a2a_out = {}
                a2a_in[0] = dramB.tile([N_CORES, 128, HL, 128], BF16, name="a2ain0")
                a2a_out[0] = dramB.tile([N_CORES, 128, HL, 128], BF16, name="a2aout0")
                for hp in (0, 2):
                    a2a_in[(1, hp)] = dramB.tile(
                        [N_CORES, 128, 2, 128], BF16, name=f"a2ain1{hp}")
                    a2a_out[(1, hp)] = dramB.tile(
                        [N_CORES, 128, 2, 128], BF16, name=f"a2aout1{hp}")

                # wo chunks (4MB, bufs=3): first three prefetched now, later
                # ones emitted lazily right after the o_proj block that frees
                # their SBUF slot (so their WAR-gated DMA triggers never sit
                # in front of collective-critical work on the sync queue).
                # Chunks 0..2 are reloaded for their hf=1 pass at the very
                # end - cheaper than holding 16MB of wo resident.
                wo_sb = {}

                def ensure_wo(ec):
                    if ec >= NECH:
                        ec -= NECH
                    w_t = wop.tile([128, H, QC], BF16, tag="wo", name=f"wo{ec}r")
                    # halves on both DMA queues in parallel: each chunk lands
                    # in half the time, so the tail's slot->DMA chain never
                    # starves the block pairs
                    h0 = ec * QC
                    nc.scalar.dma_start(out=w_t[:, :, 0:256], in_=wo[:, :, h0:h0 + 256])
                    nc.sync.dma_start(out=w_t[:, :, 256:QC], in_=wo[:, :, h0 + 256:h0 + QC])
                    wo_sb[ec] = w_t

                # only chunk 0 up front - chunks 1, 2 are paced into the qc
                # loop so their DMAs don't contend with the first AllToAll
                ensure_wo(0)

                def fire_a2a(key):
                    if key == 0:
                        hf, hsl, asl = 0, slice(0, HL), slice(0, HL)
                    else:
                        hf, hp = key
                        hsl, asl = slice(hp, hp + 2), slice(hp, hp + 2)
                    base = hf * THALF
                    for r in range(N_CORES):
                        nc.sync.dma_start(
                            out=a2a_in[key][r],
                            in_=attnT[:, hsl, base + r * 128:base + (r + 1) * 128],
                        )
                    nc.gpsimd.collective_compute(
                        "AllToAll",
                        mybir.AluOpType.bypass,
                        replica_groups=[list(range(N_CORES))],
                        ins=[a2a_in[key][:]],
                        outs=[a2a_out[key][:]],
                    )
                    nh = asl.stop - asl.start
                    for r in range(N_CORES):
                        nc.gpsimd.dma_start(
                            out=atf[hf][:, r * HL + asl.start:r * HL + asl.start + nh, :],
                            in_=a2a_out[key][r],
                        )

                def oproj_blocks(ecs_hf, pool):
                    # emit 1-3 blocks with interleaved psum chains; blocks of
                    # the same hf share every atf-head stationary load.
                    pops = [pool.tile([128, QC], F32, tag="op", name="pop")
                            for _ in ecs_hf]
                    for h in range(H):
                        for (ec, hf), pop in zip(ecs_hf, pops):
                            nc.tensor.matmul(
                                pop[:],
                                atf[hf][:, h, :],
                                wo_sb[ec][:, h, :],
                                start=(h == 0), stop=(h == H - 1),
                            )
                    for (ec, hf), pop in zip(ecs_hf, pops):
                        ost = ostg_p.tile([128, QC], BF16, tag="ost", name="ost")
                        nc.vector.tensor_copy(ost[:], pop[:])
                        nc.sync.dma_start(
                            out=out_p[hf, :, ec * QC:(ec + 1) * QC], in_=ost[:]
                        )

                pending_norm = [None]

                def flush_norm():
                    if pending_norm[0] is not None:
                        pending_norm[0]()
                        pending_norm[0] = None

                attn_pools = [
                    tc.tile_pool(name="pscp", bufs=4, space="PSUM"),
                    tc.tile_pool(name="pavp", bufs=2, space="PSUM"),
                    tc.tile_pool(name="psrp", bufs=2, space="PSUM"),
                ]
                pscp, pavp, psrp = (p.__enter__() for p in attn_pools)
                for qc in range(N_QC):
                    qsl = slice(qc * QC, (qc + 1) * QC)
                    si_lo = max(0, 4 * qc - 8)
                    sis = list(range(si_lo, 4 * qc + 4))
                    for hp in range(0, HL, 2):
                        avs, accs = [], []
                        for j in range(2):
                            avs.append(pavp.tile([128, QC], F32, tag="av", name="av"))
                            accs.append(accp.tile([128, QC], BF16, tag="acc",
                                                  name="acc"))
                        exs = {}

                        def emit_scores(si):
                            delta = qc * QC - si * 128
                            mi = MASK_IDX.get(delta)
                            for j in range(2):
                                psc = pscp.tile([128, QC], F32, tag="sc", name="psc")
                                nc.tensor.matmul(
                                    psc[:], kT[:, si * 128:(si + 1) * 128],
                                    qT[hp + j][:, qsl], start=True, stop=(mi is None),
                                )
                                if mi is not None:
                                    # additive -30000 mask folded into the
                                    # score psum group (stays on the PE; a
                                    # helper-engine mask serializes behind
                                    # the blocking collectives)
                                    nc.tensor.matmul(
                                        psc[:], ident_sb[:], mask_sb[:, mi, :],
                                        start=False, stop=True,
                                    )
                                ex = exp_p.tile([128, QC], BF16, tag="ex", name="ex")
                                nc.scalar.activation(ex[:], psc[:], AF.Exp)
                                exs[(si, j)] = ex

                        def emit_consume(si):
                            first = si == sis[0]
                            for j in range(2):
                                nc.tensor.matmul(
                                    avs[j][:], vnat[:, si, :], exs[(si, j)][:],
                                    start=first, stop=(si == sis[-1]),
                                )
                            # denominator accumulated off the PE on the DVE
                            # (gpsimd would serialize behind the blocking
                            # collective triggers)
                            for j in range(2):
                                ex = exs.pop((si, j))
                                with nc.allow_low_precision(reason="bf16 denom ok at 2e-2 gate"):
                                    if first:
                                        nc.vector.tensor_copy(accs[j][:], ex[:])
                                    else:
                                        nc.vector.tensor_add(accs[j][:], accs[j][:], ex[:])

                        # score lookahead covers the exp/mask chain
                        la = min(5, len(sis))
                        for si in sis[:la]:
                            emit_scores(si)
                        flush_norm()
                        for idx in range(la, len(sis)):
                            emit_scores(sis[idx])
                            emit_consume(sis[idx - la])
                        for si in sis[-la:]:
                            emit_consume(si)

                        def make_norm(hp=hp, avs=avs, accs=accs, qsl=qsl):
                            def _norm():
                                for j in range(2):
                                    srow = psrp.tile([1, QC], F32, tag="sr",
                                                     name="srow")
                                    nc.tensor.matmul(
                                        srow[:], onc_sb[:], accs[j][:],
                                        start=True, stop=True,
                                    )
                                    rrf = stB.tile([1, QC], F32, tag="rrf", name="rrfB")
                                    nc.vector.reciprocal_approx_fast(rrf[:], srow[:])
                                    rr = stB.tile([1, QC], BF16, tag="rr", name="rrB")
                                    with nc.allow_low_precision(reason="bf16 denom ok at 2e-2 gate"):
                                        nc.scalar.activation(rr[:], rrf[:], AF.Copy)
                                    bc = pscp.tile([128, QC], F32, tag="sc", name="bc")
                                    nc.tensor.matmul(
                                        bc[:], ones_sb[0:1, :], rr[:], start=True, stop=True
                                    )
                                    rbc = stB.tile([128, QC], BF16, tag="rbc", name="rbc")
                                    nc.vector.tensor_copy(rbc[:], bc[:])
                                    nc.vector.tensor_mul(
                                        attnT[:, hp + j, qsl], avs[j][:], rbc[:]
                                    )
                            return _norm

                        pending_norm[0] = make_norm()
                        if qc == 3 and hp == 0:
                            # first half of the hf=1 exchange overlaps the
                            # qc3-hp2 attention
                            flush_norm()
                            fire_a2a((1, 0))
                    flush_norm()
                    # fire the rows-half A2A as soon as all its heads are done
                    if qc == 0:
                        ensure_wo(1)
                    if qc == 1:
                        fire_a2a(0)
                        ensure_wo(2)
                    if qc == 3:
                        fire_a2a((1, 2))

                for p in reversed(attn_pools):
                    p.__exit__(None, None, None)
                # tail: chunks 0..2 stay resident from the attention phase
                # (zero reloads); hf=0 blocks first so the PE stays busy
                # while the hf=1 A2A drains, then ec-pairs with each freed
                # slot immediately chased by the next chunk DMA.
                with tc.tile_pool(name="popt", bufs=3, space="PSUM") as popt:
                    oproj_blocks([(0, 0), (1, 0), (2, 0)], popt)
                    oproj_blocks([(2, 1)], popt)
                    ensure_wo(3)
                    oproj_blocks([(0, 1), (1, 1)], popt)
                    ensure_wo(4)
                    ensure_wo(5)
                    oproj_blocks([(3, 0), (3, 1)], popt)
                    ensure_wo(6)
                    oproj_blocks([(4, 0), (4, 1)], popt)
                    ensure_wo(7)
                    oproj_blocks([(5, 0), (5, 1)], popt)
                    oproj_blocks([(6, 0), (6, 1)], popt)
                    oproj_blocks([(7, 0), (7, 1)], popt)

    nc.finalize()
    return nc


_NC_CACHE = None


def _get_nc():
    global _NC_CACHE
    if _NC_CACHE is None:
        _NC_CACHE = _build()
    return _NC_CACHE


def _host_inputs(positions, hidden_states, w_qkv, q_norm_w, k_norm_w, w_o):
    positions = np.asarray(positions)
    hidden_states = np.asarray(hidden_states, dtype=np.float32)
    w_qkv = np.asarray(w_qkv, dtype=np.float32)
    q_norm_w = np.asarray(q_norm_w, dtype=np.float32)
    k_norm_w = np.asarray(k_norm_w, dtype=np.float32)
    w_o = np.asarray(w_o, dtype=np.float32)

    # hidT[ki, cc, ko, tq] = hidden[cc*512 + tq, ko*128 + ki]
    hidT = np.ascontiguousarray(
        hidden_states.T.reshape(KO, 128, N_QC, QC).transpose(1, 2, 0, 3)
    ).astype(ml_dtypes.bfloat16)

    half = D // 2
    inv_freq = 1.0 / (THETA ** (np.arange(half, dtype=np.float32) / half))
    ang = positions.astype(np.float32)[:, None] * inv_freq[None, :]  # [T, 64]
    cos = np.cos(ang).T.astype(np.float32)   # [64, T]
    sin = np.sin(ang).T.astype(np.float32)
    csb = np.concatenate([cos, cos], axis=0)          # [128, T]
    snb = np.concatenate([-sin, sin], axis=0)         # [128, T]
    qwf = q_norm_w * (D ** -0.5)
    kwf = k_norm_w
    csq = (csb * qwf[:, None]).astype(ml_dtypes.bfloat16)
    snq = (snb * np.roll(qwf, -64)[:, None]).astype(ml_dtypes.bfloat16)
    csk = (csb * kwf[:, None]).astype(ml_dtypes.bfloat16)
    snk = (snb * np.roll(kwf, -64)[:, None]).astype(ml_dtypes.bfloat16)

    # additive masks (0 in-window, -30000 outside)
    mk = np.zeros((len(MASK_DELTAS), 128, QC), np.float32)
    ss = np.arange(128)[:, None]
    ttv = np.arange(QC)[None, :]
    for i, dlt in enumerate(MASK_DELTAS):
        diff = dlt + ttv - ss
        mk[i] = np.where((diff >= 0) & (diff < WINDOW), 0.0, -30000.0)
    maskm = np.ascontiguousarray(mk.transpose(1, 0, 2)).astype(ml_dtypes.bfloat16)

    # wo[ki, h, e] = w_o[h*128 + ki, e]  (full matrix, replicated)
    wo_h = np.ascontiguousarray(
        w_o.reshape(H, 128, HID).transpose(1, 0, 2)
    ).astype(ml_dtypes.bfloat16)

    onesd = np.ones((128, 128), ml_dtypes.bfloat16)
    oncd = np.ones((128, 1), ml_dtypes.bfloat16)
    identd = np.eye(128, dtype=np.float32).astype(ml_dtypes.bfloat16)

    in_maps = []
    for c in range(N_CORES):
        # c-tile order: q0..q3, k, v
        wq_c = np.concatenate(
            [
                w_qkv[:, c * HL * D:(c + 1) * HL * D],
                w_qkv[:, H * D + c * D:H * D + (c + 1) * D],
                w_qkv[:, (H + HKV) * D + c * D:(H + HKV) * D + (c + 1) * D],
            ],
            axis=1,
        )  # [4096, 768]
        # [ki, ct, ko, cj]
        wq_c = np.ascontiguousarray(
            wq_c.reshape(KO, 128, N_CT, 128).transpose(1, 2, 0, 3)
        ).astype(ml_dtypes.bfloat16)
        in_maps.append(
            {
                "hidT": hidT,
                "wq": wq_c,
                "csq": csq,
                "snq": snq,
                "csk": csk,
                "snk": snk,
                "maskm": maskm,
                "wo": wo_h,
                "onesd": onesd,
                "oncd": oncd,
                "identd": identd,
            }
        )
    return in_maps


def _assemble(results):
    out = np.empty((T, HID), np.float32)
    for c in range(N_CORES):
        r = np.asarray(results[c]["out"]).astype(np.float32)  # [2, 128, HID] bf16
        out[c * 128:(c + 1) * 128] = r[0]
        out[THALF + c * 128:THALF + (c + 1) * 128] = r[1]
    return out


def run_spmd(in_maps, trace=False, **kw):
    nc = _get_nc()
    return run_bass_kernel_spmd(nc, in_maps, list(range(N_CORES)), trace=trace, **kw)


def kernel(positions, hidden_states, w_qkv, q_norm_w, k_norm_w, w_o):
    in_maps = _host_inputs(positions, hidden_states, w_qkv, q_norm_w, k_norm_w, w_o)
    last_err = None
    for _ in range(3):
        try:
            res = run_spmd(in_maps)
            return _assemble(res.results)
        except Exception as e:  # rare transient NRT_EXEC_UNIT_UNRECOVERABLE
            last_err = e
    raise last_err


# revision 67
# speedup vs baseline: 1.0459x; 1.0459x over previous
"""Bass/Trainium2 kernel for nn_ExaoneMoEAttention (sliding-window GQA attention).

Strategy (8 NeuronCores, tensor-parallel over heads for QKV+attention,
token-sharded o_proj via AllToAll).  All GEMMs bf16: fp8/DoubleRow fails the
2e-2 max-rel gate because quantization noise concentrates in the first ~64
token rows (attention there averages few keys, so |attn| ~ |v| and the 3%
fp8 mantissa error lands directly on the output rows that set max|out|).
The per-matmul issue floor is ~260ns regardless of dtype or size, so the
design minimizes matmul instruction COUNT and keeps every engine's queue
free of cross-engine blockers:
  - core c owns q heads 4c..4c+3 and kv head c (w_qkv column shard [4096, 768]).
  - Phase A (QKV proj, transposed): chunk-major over four 512-column hid
    chunks (double-buffered 4MB DMAs); per chunk, ci PAIRS run as two
    interleaved psum chains so matmul durations overlap.  Startup DMAs are
    split fine (first pair gates on ~2MB).  RMSNorm/RoPE post-ops trail in
    3 lag stages: ones-matmul partition-reduce of sq=ps^2/D (scale folded
    into the Square activation, eps dropped as negligible), sqrt on ACT,
    reciprocal_approx_fast on DVE, Copy-downcast on ACT, ones-matmul
    broadcast, rope as two half-tile muls with norm weight and softmax
    scale folded into host cos/sin tables.  v is PE-transposed.
  - Phase B attention: scores = kT-block.T @ qT (N=512) with the additive
    -30000 mask folded into the score psum group (a helper-engine mask
    serializes behind the blocking collectives); exp on ACT (no max
    subtraction, |score| <= sqrt(D)); softmax denominator accumulated as
    bf16 tensor_adds on the DVE and partition-reduced by a single
    ones-matmul per head-chunk in the (deferred) normalization; two GQA
    heads pipelined per group, 4-deep score lookahead.
  - A2A schedule: hf=0 exchanged as ONE merged 1MB AllToAll (hides under
    qc2/3); hf=1 split per head-pair, the first fired under qc3-hp2 so only
    a 512KB exchange remains after attention ends.  The o_proj tail opens
    with three hf=0 blocks to cover that drain.
  - o_proj: each core owns 256 token rows with all 32 heads; out =
    attnT_full.T @ w_o against streamed bf16 w_o (4MB chunks, bufs=3,
    alternating DMA queues, chunks 0..2 resident from attention so nothing
    reloads).  Tail blocks are emitted in PAIRS/TRIPLES sharing each
    atf-head stationary with interleaved psum chains.  Output is directly
    row-sharded - host assembly is pure concatenation.
  - a tiny dummy AllToAll at kernel start absorbs the ~50us core-start skew.

Measured on 8 axon-tunneled trn2 cores: 577us (baseline 717us), max rel err
0.0043 vs the fp32 reference (gate 2e-2).
"""

import ml_dtypes
import numpy as np

import concourse.bass as bass
import concourse.mybir as mybir
import concourse.tile as tile
from concourse import bacc
from concourse.bass_utils import run_bass_kernel_spmd

F32 = mybir.dt.float32
BF16 = mybir.dt.bfloat16
AF = mybir.ActivationFunctionType

N_CORES = 8
T = 2048
HID = 4096
H = 32
HKV = 8
D = 128
EPS = 1e-5
THETA = 1e6
WINDOW = 1024

HL = H // N_CORES          # 4 local q heads
NT = T // 128              # 16 s tiles
KO = HID // 128            # 32 k-subtiles in projection
QC = 512                   # q chunk in attention phase
N_QC = T // QC             # 4
THALF = T // 2
NECH = HID // QC           # 8 o_proj e-chunks

MASK_DELTAS = [0, -128, -256, -384, 640, 768, 896, 1024]
MASK_IDX = {d: i for i, d in enumerate(MASK_DELTAS)}

# c-tile order within the per-core w_qkv shard: q0..q3, k, v
N_CT = HL + 2


def _build():
    nc = bacc.Bacc(num_devices=N_CORES)

    hidT = nc.declare_dram_parameter("hidT", [128, N_QC, KO, QC], BF16, isOutput=False)
    wq = nc.declare_dram_parameter("wq", [128, N_CT, KO, 128], BF16, isOutput=False)
    csq = nc.declare_dram_parameter("csq", [128, T], BF16, isOutput=False)
    snq = nc.declare_dram_parameter("snq", [128, T], BF16, isOutput=False)
    csk = nc.declare_dram_parameter("csk", [128, T], BF16, isOutput=False)
    snk = nc.declare_dram_parameter("snk", [128, T], BF16, isOutput=False)
    maskm = nc.declare_dram_parameter("maskm", [128, len(MASK_DELTAS), QC], BF16, isOutput=False)
    wo = nc.declare_dram_parameter("wo", [128, H, HID], BF16, isOutput=False)
    onesd = nc.declare_dram_parameter("onesd", [128, 128], BF16, isOutput=False)
    oncd = nc.declare_dram_parameter("oncd", [128, 1], BF16, isOutput=False)
    identd = nc.declare_dram_parameter("identd", [128, 128], BF16, isOutput=False)
    out_p = nc.declare_dram_parameter("out", [2, 128, HID], BF16, isOutput=True)

    with tile.TileContext(nc) as tc:
        with (
            tc.tile_pool(name="pers", bufs=1) as pers,
            tc.tile_pool(name="dram0", bufs=1, space="DRAM") as dram0,
        ):
            qT = [pers.tile([128, T], BF16, name=f"qT{h}") for h in range(HL)]
            kT = pers.tile([128, T], BF16)
            vnat = pers.tile([128, NT, 128], BF16)
            attnT = pers.tile([128, HL, T], BF16)
            ones_sb = pers.tile([128, 128], BF16)
            nc.sync.dma_start(out=ones_sb[:], in_=onesd[:])
            onc_sb = pers.tile([128, 1], BF16)
            nc.sync.dma_start(out=onc_sb[:], in_=oncd[:])
            ident_sb = pers.tile([128, 128], BF16)
            nc.sync.dma_start(out=ident_sb[:], in_=identd[:])
            # attention masks preloaded on the idle gpsimd queue so the first
            # masked score matmul never waits on this 0.5MB
            mask_sb = pers.tile([128, len(MASK_DELTAS), QC], BF16)
            nc.gpsimd.dma_start(out=mask_sb[:], in_=maskm[:])

            # tiny dummy collective: absorbs multi-core start skew so the
            # real AllToAlls later don't pay it (first collective waits for
            # the slowest core).
            dumm_i = dram0.tile([N_CORES, 16], BF16, name="dummi")
            dumm_o = dram0.tile([N_CORES, 16], BF16, name="dummo")
            nc.sync.dma_start(out=dumm_i[0:1, :], in_=onesd[0:1, 0:16])
            nc.gpsimd.collective_compute(
                "AllToAll",
                mybir.AluOpType.bypass,
                replica_groups=[list(range(N_CORES))],
                ins=[dumm_i[:]],
                outs=[dumm_o[:]],
            )

            # ---------------- Phase A: transposed QKV projection --------------
            with (
                tc.tile_pool(name="tabp", bufs=1) as tabp,
                tc.tile_pool(name="hidp", bufs=1) as hidp,
                tc.tile_pool(name="wqp", bufs=1) as wqp,
                tc.tile_pool(name="stg", bufs=2) as stg,
                tc.tile_pool(name="projp", bufs=2, space="PSUM") as projp,
                tc.tile_pool(name="varp", bufs=1, space="PSUM") as varp,
                tc.tile_pool(name="bcp", bufs=1, space="PSUM") as bcp,
            ):
                # wq resident; hid double-buffered by 4MB column chunk so the
                # first matmul pair gates on wq[ci 0,1] + chunk 0 (~6MB).
                # ci PAIRS run as two interleaved psum chains per chunk so
                # consecutive matmul durations overlap.
                hid_sb = hidp.tile([128, 2, KO, QC], BF16, name="hid_sb")
                wq_sb = wqp.tile([128, N_CT, KO, 128], BF16, name="wq_sb")
                ci_pairs = ((0, 1), (HL, HL + 1), (2, 3))
                # priority order, split fine so the first matmul pair gates
                # on ~2MB (wq[0,1] ko 0..15 + hid chunk 0 ko 0..7) and the
                # ko loop chases the in-flight rest
                nc.scalar.dma_start(out=wq_sb[:, 0, 0:16], in_=wq[:, 0, 0:16])
                nc.scalar.dma_start(out=wq_sb[:, 1, 0:16], in_=wq[:, 1, 0:16])
                for kb in range(4):
                    ksl = slice(kb * 8, (kb + 1) * 8)
                    nc.sync.dma_start(out=hid_sb[:, 0, ksl], in_=hidT[:, 0, ksl])
                nc.scalar.dma_start(out=wq_sb[:, 0, 16:32], in_=wq[:, 0, 16:32])
                nc.scalar.dma_start(out=wq_sb[:, 1, 16:32], in_=wq[:, 1, 16:32])
                for ci in (HL, HL + 1, 2, 3):
                    nc.scalar.dma_start(out=wq_sb[:, ci], in_=wq[:, ci])
                nc.sync.dma_start(out=hid_sb[:, 1], in_=hidT[:, 1])

                cs_sb = {}
                for nm, src in (("csq", csq), ("snq", snq), ("csk", csk), ("snk", snk)):
                    t_ = tabp.tile([128, T], BF16, name=nm)
                    nc.scalar.dma_start(out=t_[:], in_=src[:])
                    cs_sb[nm] = t_

                # posts run in 3 lag stages so no PE instruction ever waits
                # on ACT/DVE work emitted in the same flush (which stalls the
                # in-order PE queue): stage1 evacuates psum (ACT/DVE only),
                # stage2 does the var matmul + sqrt + recip, stage3 the
                # broadcast matmul + rope muls (all inputs one flush old).
                pend = {1: [], 2: [], 3: []}

                def flush_stages(keep):
                    for lag in (1, 2, 3):
                        while len(pend[lag]) > keep.get(lag, 0):
                            pend[lag].pop(0)()

                for cc in range(N_QC):
                    buf = cc % 2
                    cols = slice(cc * QC, (cc + 1) * QC)
                    for cia, cib in ci_pairs:
                        ps = {}
                        for i, ci in enumerate((cia, cib)):
                            ps[ci] = projp.tile([128, QC], F32, tag=f"pj{i}",
                                                name=f"ps{i}")
                        for ko in range(KO):
                            for ci in (cia, cib):
                                nc.tensor.matmul(
                                    ps[ci][:], wq_sb[:, ci, ko],
                                    hid_sb[:, buf, ko, :],
                                    start=(ko == 0), stop=(ko == KO - 1),
                                )

                        evs, sqs, rrs = {}, {}, {}
                        for ci in (cia, cib):
                            evs[ci] = stg.tile([128, QC], BF16, tag="ev",
                                               name="ev", bufs=6)
                            if ci != HL + 1:
                                sqs[ci] = stg.tile([128, QC], BF16, tag="sq",
                                                   name="sq", bufs=4)
                                rrs[ci] = stg.tile([1, QC], BF16, tag="rr",
                                                   name="rr", bufs=4)

                        def st1(ps=ps, evs=evs, sqs=sqs):
                            for ci, p in ps.items():
                                if ci in sqs:
                                    # scale folded in: sq = ps^2/D, so the
                                    # ones-matmul reduce gives mean(x^2)
                                    # directly (eps=1e-5 is negligible vs
                                    # var~1.6 and is dropped)
                                    nc.scalar.activation(
                                        sqs[ci][:], p[:], AF.Square,
                                        scale=D ** -0.5,
                                    )
                                nc.vector.tensor_copy(evs[ci][:], p[:])
                        pend[1].append(st1)

                        def st2(cc=cc, evs=evs, sqs=sqs, rrs=rrs):
                            for ci in evs:
                                if ci == HL + 1:
                                    for b in range(4):
                                        si = cc * 4 + b
                                        tp = varp.tile([128, 128], BF16, tag="vt",
                                                       name="tp", bufs=2)
                                        nc.tensor.transpose(
                                            tp[:], evs[ci][:, b * 128:(b + 1) * 128],
                                            ident_sb[:],
                                        )
                                        nc.vector.tensor_copy(vnat[:, si, :], tp[:])
                                    continue
                                vr = varp.tile([1, QC], F32, tag="var", name="vr", bufs=1)
                                nc.tensor.matmul(
                                    vr[:], onc_sb[:], sqs[ci][:],
                                    start=True, stop=True,
                                )
                                sdr = stg.tile([1, QC], F32, tag="sdr", name="sdr")
                                nc.scalar.activation(sdr[:], vr[:], AF.Sqrt)
                                rrf = stg.tile([1, QC], F32, tag="rrf", name="rrf")
                                nc.vector.reciprocal_approx_fast(rrf[:], sdr[:])
                                with nc.allow_low_precision(reason="bf16 rstd ok at 2e-2 gate"):
                                    nc.scalar.activation(rrs[ci][:], rrf[:], AF.Copy)
                        pend[2].append(st2)

                        def st3(cols=cols, evs=evs, rrs=rrs):
                            for ci in evs:
                                if ci == HL + 1:
                                    continue
                                if ci == HL:
                                    cs_t, sn_t, dst = cs_sb["csk"], cs_sb["snk"], kT
                                else:
                                    cs_t, sn_t, dst = cs_sb["csq"], cs_sb["snq"], qT[ci]
                                rbc = bcp.tile([128, QC], F32, tag="bc", name="rbc")
                                nc.tensor.matmul(
                                    rbc[:], ones_sb[0:1, :], rrs[ci][:],
                                    start=True, stop=True,
                                )
                                t1 = stg.tile([128, QC], BF16, tag="t1", name="t1")
                                nc.vector.tensor_mul(t1[:], evs[ci][:], cs_t[:, cols])
                                sw = stg.tile([128, QC], BF16, tag="sw", name="sw")
                                nc.vector.tensor_copy(sw[0:64, :], evs[ci][64:128, :])
                                nc.vector.tensor_copy(sw[64:128, :], evs[ci][0:64, :])
                                t2 = stg.tile([128, QC], BF16, tag="t2", name="t2")
                                nc.vector.tensor_mul(t2[:], sw[:], sn_t[:, cols])
                                nc.vector.tensor_add(t1[:], t1[:], t2[:])
                                nc.vector.tensor_mul(dst[:, cols], t1[:], rbc[:])
                        pend[3].append(st3)

                        flush_stages({1: 1, 2: 2, 3: 3})
                    if cc + 2 < N_QC:
                        # prefetch chunk cc+2 into the buffer chunk cc just
                        # finished reading (WAR on the emitted matmuls)
                        nc.sync.dma_start(out=hid_sb[:, buf], in_=hidT[:, cc + 2])
                flush_stages({})

            # ---------------- Phase B: attention + A2A + o_proj ---------------
            with (
                tc.tile_pool(name="pB", bufs=1) as pB,
                tc.tile_pool(name="wop", bufs=3) as wop,
                tc.tile_pool(name="exp", bufs=10) as exp_p,
                tc.tile_pool(name="accp", bufs=4) as accp,
                tc.tile_pool(name="stB", bufs=3) as stB,
                tc.tile_pool(name="ostg", bufs=2) as ostg_p,
                tc.tile_pool(name="dramB", bufs=1, space="DRAM") as dramB,
            ):
                atf = [pB.tile([128, H, 128], BF16, name=f"atf{h}") for h in range(2)]

                # hf=0: ONE merged A2A (all 4 local heads, 1MB) - it hides
                # fully under qc2/qc3 attention.  hf=1: split per head-pair so
                # the first half overlaps qc3-hp2 attention and only a small
                # 512KB exchange remains exposed after attention ends.
                a2a_in = {}
                a2a_out = {}
                a2a_in[0] = dramB.tile([N_CORES, 128, HL, 128], BF16, name="a2ain0")
                a2a_out[0] = dramB.tile([N_CORES, 128, HL, 128], BF16, name="a2aout0")
                for hp in (0, 2):
                    a2a_in[(1, hp)] = dramB.tile(
                        [N_CORES, 128, 2, 128], BF16, name=f"a2ain1{hp}")
                    a2a_out[(1, hp)] = dramB.tile(
                        [N_CORES, 128, 2, 128], BF16, name=f"a2aout1{hp}")

                # wo chunks (4MB, bufs=3): first three prefetched now, later
                # ones emitted lazily right after the o_proj block that frees
                # their SBUF slot (so their WAR-gated DMA triggers never sit
                # in front of collective-critical work on the sync queue).
                # Chunks 0..2 are reloaded for their hf=1 pass at the very
                # end - cheaper than holding 16MB of wo resident.
                wo_sb = {}

                def ensure_wo(ec):
                    if ec >= NECH:
                        ec -= NECH
                    w_t = wop.tile([128, H, QC], BF16, tag="wo", name=f"wo{ec}r")
                    # alternate queues so two chunk DMAs stream concurrently
                    eng = nc.scalar if ec % 2 == 0 else nc.sync
                    eng.dma_start(out=w_t[:], in_=wo[:, :, ec * QC:(ec + 1) * QC])
                    wo_sb[ec] = w_t

                # only chunk 0 up front - chunks 1, 2 are paced into the qc
                # loop so their DMAs don't contend with the first AllToAll
                ensure_wo(0)

                def fire_a2a(key):
                    if key == 0:
                        hf, hsl, asl = 0, slice(0, HL), slice(0, HL)
                    else:
                        hf, hp = key
                        hsl, asl = slice(hp, hp + 2), slice(hp, hp + 2)
                    base = hf * THALF
                    for r in range(N_CORES):
                        nc.sync.dma_start(
                            out=a2a_in[key][r],
                            in_=attnT[:, hsl, base + r * 128:base + (r + 1) * 128],
                        )
                    nc.gpsimd.collective_compute(
                        "AllToAll",
                        mybir.AluOpType.bypass,
                        replica_groups=[list(range(N_CORES))],
                        ins=[a2a_in[key][:]],
                        outs=[a2a_out[key][:]],
                    )
                    nh = asl.stop - asl.start
                    for r in range(N_CORES):
                        nc.gpsimd.dma_start(
                            out=atf[hf][:, r * HL + asl.start:r * HL + asl.start + nh, :],
                            in_=a2a_out[key][r],
                        )

                def oproj_blocks(ecs_hf, pool):
                    # emit 1-3 blocks with interleaved psum chains; blocks of
                    # the same hf share every atf-head stationary load.
                    pops = [pool.tile([128, QC], F32, tag="op", name="pop")
                            for _ in ecs_hf]
                    for h in range(H):
                        for (ec, hf), pop in zip(ecs_hf, pops):
                            nc.tensor.matmul(
                                pop[:],
                                atf[hf][:, h, :],
                                wo_sb[ec][:, h, :],
                                start=(h == 0), stop=(h == H - 1),
                            )
                    for (ec, hf), pop in zip(ecs_hf, pops):
                        ost = ostg_p.tile([128, QC], BF16, tag="ost", name="ost")
                        nc.vector.tensor_copy(ost[:], pop[:])
                        nc.sync.dma_start(
                            out=out_p[hf, :, ec * QC:(ec + 1) * QC], in_=ost[:]
                        )

                pending_norm = [None]

                def flush_norm():
                    if pending_norm[0] is not None:
                        pending_norm[0]()
                        pending_norm[0] = None

                attn_pools = [
                    tc.tile_pool(name="pscp", bufs=4, space="PSUM"),
                    tc.tile_pool(name="pavp", bufs=2, space="PSUM"),
                    tc.tile_pool(name="psrp", bufs=2, space="PSUM"),
                ]
                pscp, pavp, psrp = (p.__enter__() for p in attn_pools)
                for qc in range(N_QC):
                    qsl = slice(qc * QC, (qc + 1) * QC)
                    si_lo = max(0, 4 * qc - 8)
                    sis = list(range(si_lo, 4 * qc + 4))
                    for hp in range(0, HL, 2):
                        avs, accs = [], []
                        for j in range(2):
                            avs.append(pavp.tile([128, QC], F32, tag="av", name="av"))
                            accs.append(accp.tile([128, QC], BF16, tag="acc",
                                                  name="acc"))
                        exs = {}

                        def emit_scores(si):
                            delta = qc * QC - si * 128
                            mi = MASK_IDX.get(delta)
                            for j in range(2):
                                psc = pscp.tile([128, QC], F32, tag="sc", name="psc")
                                nc.tensor.matmul(
                                    psc[:], kT[:, si * 128:(si + 1) * 128],
                                    qT[hp + j][:, qsl], start=True, stop=(mi is None),
                                )
                                if mi is not None:
                                    # additive -30000 mask folded into the
                                    # score psum group (stays on the PE; a
                                    # helper-engine mask serializes behind
                                    # the blocking collectives)
                                    nc.tensor.matmul(
                                        psc[:], ident_sb[:], mask_sb[:, mi, :],
                                        start=False, stop=True,
                                    )
                                ex = exp_p.tile([128, QC], BF16, tag="ex", name="ex")
                                nc.scalar.activation(ex[:], psc[:], AF.Exp)
                                exs[(si, j)] = ex

                        def emit_consume(si):
                            first = si == sis[0]
                            for j in range(2):
                                nc.tensor.matmul(
                                    avs[j][:], vnat[:, si, :], exs[(si, j)][:],
                                    start=first, stop=(si == sis[-1]),
                                )
                            # denominator accumulated off the PE on the DVE
                            # (gpsimd would serialize behind the blocking
                            # collective triggers)
                            for j in range(2):
                                ex = exs.pop((si, j))
                                with nc.allow_low_precision(reason="bf16 denom ok at 2e-2 gate"):
                                    if first:
                                        nc.vector.tensor_copy(accs[j][:], ex[:])
                                    else:
                                        nc.vector.tensor_add(accs[j][:], accs[j][:], ex[:])

                        # score lookahead covers the exp/mask chain
                        la = min(4, len(sis))
                        for si in sis[:la]:
                            emit_scores(si)
                        flush_norm()
                        for idx in range(la, len(sis)):
                            emit_scores(sis[idx])
                            emit_consume(sis[idx - la])
                        for si in sis[-la:]:
                            emit_consume(si)

                        def make_norm(hp=hp, avs=avs, accs=accs, qsl=qsl):
                            def _norm():
                                for j in range(2):
                                    srow = psrp.tile([1, QC], F32, tag="sr",
                                                     name="srow")
                                    nc.tensor.matmul(
                                        srow[:], onc_sb[:], accs[j][:],
                                        start=True, stop=True,
                                    )
                                    rrf = stB.tile([1, QC], F32, tag="rrf", name="rrfB")
                                    nc.vector.reciprocal_approx_fast(rrf[:], srow[:])
                                    rr = stB.tile([1, QC], BF16, tag="rr", name="rrB")
                                    with nc.allow_low_precision(reason="bf16 denom ok at 2e-2 gate"):
                                        nc.scalar.activation(rr[:], rrf[:], AF.Copy)
                                    bc = pscp.tile([128, QC], F32, tag="sc", name="bc")
                                    nc.tensor.matmul(
                                        bc[:], ones_sb[0:1, :], rr[:], start=True, stop=True
                                    )
                                    rbc = stB.tile([128, QC], BF16, tag="rbc", name="rbc")
                                    nc.vector.tensor_copy(rbc[:], bc[:])
                                    nc.vector.tensor_mul(
                                        attnT[:, hp + j, qsl], avs[j][:], rbc[:]
                                    )
                            return _norm

                        pending_norm[0] = make_norm()
                        if qc == 3 and hp == 0:
                            # first half of the hf=1 exchange overlaps the
                            # qc3-hp2 attention
                            flush_norm()
                            fire_a2a((1, 0))
                    flush_norm()
                    # fire the rows-half A2A as soon as all its heads are done
                    if qc == 0:
                        ensure_wo(1)
                    if qc == 1:
                        fire_a2a(0)
                        ensure_wo(2)
                    if qc == 3:
                        fire_a2a((1, 2))

                for p in reversed(attn_pools):
                    p.__exit__(None, None, None)
                # tail: chunks 0..2 stay resident from the attention phase
                # (zero reloads); hf=0 blocks first so the PE stays busy
                # while the hf=1 A2A drains, then ec-pairs with each freed
                # slot immediately chased by the next chunk DMA.
                with tc.tile_pool(name="popt", bufs=3, space="PSUM") as popt:
                    oproj_blocks([(0, 0), (1, 0), (2, 0)], popt)
                    oproj_blocks([(2, 1)], popt)
                    ensure_wo(3)
                    oproj_blocks([(0, 1), (1, 1)], popt)
                    ensure_wo(4)
                    ensure_wo(5)
                    oproj_blocks([(3, 0), (3, 1)], popt)
                    ensure_wo(6)
                    oproj_blocks([(4, 0), (4, 1)], popt)
                    ensure_wo(7)
                    oproj_blocks([(5, 0), (5, 1)], popt)
                    oproj_blocks([(6, 0), (6, 1)], popt)
                    oproj_blocks([(7, 0), (7, 1)], popt)

    nc.finalize()
    return nc


_NC_CACHE = None


def _get_nc():
    global _NC_CACHE
    if _NC_CACHE is None:
        _NC_CACHE = _build()
    return _NC_CACHE


def _host_inputs(positions, hidden_states, w_qkv, q_norm_w, k_norm_w, w_o):
    positions = np.asarray(positions)
    hidden_states = np.asarray(hidden_states, dtype=np.float32)
    w_qkv = np.asarray(w_qkv, dtype=np.float32)
    q_norm_w = np.asarray(q_norm_w, dtype=np.float32)
    k_norm_w = np.asarray(k_norm_w, dtype=np.float32)
    w_o = np.asarray(w_o, dtype=np.float32)

    # hidT[ki, cc, ko, tq] = hidden[cc*512 + tq, ko*128 + ki]
    hidT = np.ascontiguousarray(
        hidden_states.T.reshape(KO, 128, N_QC, QC).transpose(1, 2, 0, 3)
    ).astype(ml_dtypes.bfloat16)

    half = D // 2
    inv_freq = 1.0 / (THETA ** (np.arange(half, dtype=np.float32) / half))
    ang = positions.astype(np.float32)[:, None] * inv_freq[None, :]  # [T, 64]
    cos = np.cos(ang).T.astype(np.float32)   # [64, T]
    sin = np.sin(ang).T.astype(np.float32)
    csb = np.concatenate([cos, cos], axis=0)          # [128, T]
    snb = np.concatenate([-sin, sin], axis=0)         # [128, T]
    qwf = q_norm_w * (D ** -0.5)
    kwf = k_norm_w
    csq = (csb * qwf[:, None]).astype(ml_dtypes.bfloat16)
    snq = (snb * np.roll(qwf, -64)[:, None]).astype(ml_dtypes.bfloat16)
    csk = (csb * kwf[:, None]).astype(ml_dtypes.bfloat16)
    snk = (snb * np.roll(kwf, -64)[:, None]).astype(ml_dtypes.bfloat16)

    # additive masks (0 in-window, -30000 outside)
    mk = np.zeros((len(MASK_DELTAS), 128, QC), np.float32)
    ss = np.arange(128)[:, None]
    ttv = np.arange(QC)[None, :]
    for i, dlt in enumerate(MASK_DELTAS):
        diff = dlt + ttv - ss
        mk[i] = np.where((diff >= 0) & (diff < WINDOW), 0.0, -30000.0)
    maskm = np.ascontiguousarray(mk.transpose(1, 0, 2)).astype(ml_dtypes.bfloat16)

    # wo[ki, h, e] = w_o[h*128 + ki, e]  (full matrix, replicated)
    wo_h = np.ascontiguousarray(
        w_o.reshape(H, 128, HID).transpose(1, 0, 2)
    ).astype(ml_dtypes.bfloat16)

    onesd = np.ones((128, 128), ml_dtypes.bfloat16)
    oncd = np.ones((128, 1), ml_dtypes.bfloat16)
    identd = np.eye(128, dtype=np.float32).astype(ml_dtypes.bfloat16)

    in_maps = []
    for c in range(N_CORES):
        # c-tile order: q0..q3, k, v
        wq_c = np.concatenate(
            [
                w_qkv[:, c * HL * D:(c + 1) * HL * D],
                w_qkv[:, H * D + c * D:H * D + (c + 1) * D],
                w_qkv[:, (H + HKV) * D + c * D:(H + HKV) * D + (c + 1) * D],
            ],
            axis=1,
        )  # [4096, 768]
        # [ki, ct, ko, cj]
        wq_c = np.ascontiguousarray(
            wq_c.reshape(KO, 128, N_CT, 128).transpose(1, 2, 0, 3)
        ).astype(ml_dtypes.bfloat16)
        in_maps.append(
            {
                "hidT": hidT,
                "wq": wq_c,
                "csq": csq,
                "snq": snq,
                "csk": csk,
                "snk": snk,
                "maskm": maskm,
                "wo": wo_h,
                "onesd": onesd,
                "oncd": oncd,
                "identd": identd,
            }
        )
    return in_maps


def _assemble(results):
    out = np.empty((T, HID), np.float32)
    for c in range(N_CORES):
        r = np.asarray(results[c]["out"]).astype(np.float32)  # [2, 128, HID] bf16
        out[c * 128:(c + 1) * 128] = r[0]
        out[THALF + c * 128:THALF + (c + 1) * 128] = r[1]
    return out


def run_spmd(in_maps, trace=False, **kw):
    nc = _get_nc()
    return run_bass_kernel_spmd(nc, in_maps, list(range(N_CORES)), trace=trace, **kw)


def kernel(positions, hidden_states, w_qkv, q_norm_w, k_norm_w, w_o):
    in_maps = _host_inputs(positions, hidden_states, w_qkv, q_norm_w, k_norm_w, w_o)
    last_err = None
    for _ in range(3):
        try:
            res = run_spmd(in_maps)
            return _assemble(res.results)
        except Exception as e:  # rare transient NRT_EXEC_UNIT_UNRECOVERABLE
            last_err = e
    raise last_err
